# revision 1
# baseline (speedup 1.0000x reference)
"""Trainium2 Bass kernel: receptive-field one-hot time encoder.

reference semantics:
    t_spike[n, b] = clip(int(scaling[n] * |x[b] - center[n]|), 0, T-1)
    out[t, b, n]  = 1.0 if t == t_spike[n, b] else 0.0       # [T, B, N] f32

Data-parallel over 8 cores: x and the output batch axis are sharded into 8
contiguous blocks of B_c = 16384; center/scaling are replicated.

Per-core layout: x viewed as [128 partitions, 128 cols]; all per-(b, n)
quantities live in [128, 2048] tiles with free index f*16+n, which makes each
output t-slice a single fully contiguous 1 MiB DRAM store ([T, B_c, N] with
b = p*128+f).

Pipeline (all DVE, broadcast 0-stride APs for the per-n vectors):
  per f-chunk c (4 chunks of 32 cols):
    diff = x - center       (tensor_tensor, broadcast APs)
    u    = |diff|           (ACT Abs)
    v    = u * scaling      (tensor_tensor)
    t=0  one-hot = is_lt(v, 1)   -- computable pre-floor: floor(v)==0 <=> v<1
    t=63 one-hot = is_ge(v, 63)  -- and the clip: floor(v)>=63 <=> v>=63
    floor(v) robust to convert rounding (sim truncates, HW rounds-to-nearest):
        r = f32(i32(v)); tsp = r - (r > v)
    ramp t=1..4 one-hots on this chunk   (keeps the DMA stream fed during prep)
  then t=5..62 full-width: one tensor_scalar is_equal + one 1 MiB DMA each.

The one-hot compares are exact: v is an exact f32 product when compared, tsp
is an exact small integer in f32, so outputs match the jax reference bit-for-
bit (relative error 0.0, verified on HW).
"""

import sys

if "/opt/trn_rl_repo" not in sys.path:
    sys.path.insert(0, "/opt/trn_rl_repo")

import numpy as np

import concourse.tile as tile
from concourse import bacc, mybir
from concourse.alu_op_type import AluOpType
from concourse.bass_utils import run_bass_kernel_spmd

B, N, T = 131072, 16, 64
NCORES = 8
BC = B // NCORES  # 16384 per core
P = 128
F = BC // P  # 128 x-columns per partition
FN = F * N  # 2048 free elems per output row

N_CHUNKS = 4
RAMP_K = 6
OUT_BUFS = 8

f32 = mybir.dt.float32
i32 = mybir.dt.int32

_cached = {}


def build_program():
    nc = bacc.Bacc(
        "TRN2", target_bir_lowering=False, debug=False, enable_asserts=False
    )
    # packed input: [:, :N]=center, [:, N:2N]=scaling, [:, 2N:]=x
    pk_d = nc.dram_tensor("packed", [P, 2 * N + F], f32, kind="ExternalInput").ap()
    out_d = nc.dram_tensor("out", [T, P, FN], f32, kind="ExternalOutput").ap()

    fc = F // N_CHUNKS
    wc = fc * N

    with tile.TileContext(nc) as tc:
        with (
            tc.tile_pool(name="prep", bufs=1) as prep,
            tc.tile_pool(name="pp", bufs=2) as pp,
            tc.tile_pool(name="outp", bufs=OUT_BUFS) as outp,
        ):
            pk_sb = prep.tile([P, 2 * N + F], f32)
            v = prep.tile([P, FN], f32)
            tsp = prep.tile([P, FN], f32)

            nc.sync.dma_start(pk_sb[:, : 2 * N + fc], pk_d[:, : 2 * N + fc])
            nc.sync.dma_start(pk_sb[:, 2 * N + fc :], pk_d[:, 2 * N + fc :])

            cen_b = (
                pk_sb[:, 0:N]
                .rearrange("p (f n) -> p f n", f=1)
                .broadcast_to([P, fc, N])
            )
            sca_b = (
                pk_sb[:, N : 2 * N]
                .rearrange("p (f n) -> p f n", f=1)
                .broadcast_to([P, fc, N])
            )

            rfs = {}

            def chunk_head(c):
                xs = slice(2 * N + c * fc, 2 * N + (c + 1) * fc)
                ws = slice(c * wc, (c + 1) * wc)
                x_b = pk_sb[:, xs].broadcast_to([P, fc, N])
                diff = prep.tile([P, wc], f32, tag="diff")
                nc.vector.tensor_tensor(
                    diff[:].rearrange("p (f n) -> p f n", n=N),
                    x_b,
                    cen_b,
                    op=AluOpType.subtract,
                )
                u = prep.tile([P, wc], f32, tag="u")
                nc.scalar.activation(u[:], diff[:], mybir.ActivationFunctionType.Abs)
                nc.vector.tensor_tensor(
                    v[:, ws].rearrange("p (f n) -> p f n", n=N),
                    u[:].rearrange("p (f n) -> p f n", n=N),
                    sca_b,
                    op=AluOpType.mult,
                )
                o0 = outp.tile([P, wc], f32, tag="onehot_r")
                nc.vector.tensor_scalar(
                    o0[:], v[:, ws], 1.0, None, op0=AluOpType.is_lt
                )
                nc.sync.dma_start(out_d[0, :, ws], o0[:])
                o63 = outp.tile([P, wc], f32, tag="onehot_r")
                nc.vector.tensor_scalar(
                    o63[:], v[:, ws], float(T - 1), None, op0=AluOpType.is_ge
                )
                nc.sync.dma_start(out_d[T - 1, :, ws], o63[:])
                # converts routed off DVE (ACT + GPSIMD); the r-(r>v) fix
                # makes any convert rounding mode exact
                ri = pp.tile([P, wc], i32, tag="ri")
                rf = pp.tile([P, wc], f32, tag="rf")
                nc.scalar.activation(
                    ri[:], v[:, ws], mybir.ActivationFunctionType.Copy
                )
                nc.gpsimd.tensor_copy(rf[:], ri[:])
                rfs[c] = rf

            def chunk_tail(c):
                # floor-fix completion + ramped early t's, emitted one chunk
                # behind chunk_head so the ACT/GPSIMD convert latency of
                # chunk c hides behind chunk c+1's DVE prep
                ws = slice(c * wc, (c + 1) * wc)
                rf = rfs.pop(c)
                corr = prep.tile([P, wc], f32, tag="corr")
                nc.vector.tensor_tensor(
                    corr[:], rf[:], v[:, ws], op=AluOpType.is_gt
                )
                nc.vector.tensor_tensor(
                    tsp[:, ws], rf[:], corr[:], op=AluOpType.subtract
                )
                for t in range(1, 1 + RAMP_K):
                    o = outp.tile([P, wc], f32, tag="onehot_r")
                    nc.vector.tensor_scalar(
                        o[:], tsp[:, ws], float(t), None, op0=AluOpType.is_equal
                    )
                    nc.sync.dma_start(out_d[t, :, ws], o[:])

            for c in range(N_CHUNKS):
                chunk_head(c)
                if c >= 1:
                    chunk_tail(c - 1)
            chunk_tail(N_CHUNKS - 1)

            for t in range(1 + RAMP_K, T - 1):
                o = outp.tile([P, FN], f32, tag="onehot")
                nc.vector.tensor_scalar(
                    o[:], tsp[:], float(t), None, op0=AluOpType.is_equal
                )
                nc.sync.dma_start(out_d[t], o[:])

    nc.compile()
    return nc


def _get_program():
    if "nc" not in _cached:
        _cached["nc"] = build_program()
    return _cached["nc"]


def kernel(x, center, scaling, time_steps):
    assert int(time_steps) == T
    x = np.ascontiguousarray(np.asarray(x, dtype=np.float32)).reshape(
        NCORES, P, F
    )
    cen = np.asarray(center, np.float32).reshape(N)
    sca = np.asarray(scaling, np.float32).reshape(N)
    packed = np.empty((NCORES, P, 2 * N + F), np.float32)
    packed[:, :, 0:N] = cen
    packed[:, :, N : 2 * N] = sca
    packed[:, :, 2 * N :] = x
    in_maps = [{"packed": packed[c]} for c in range(NCORES)]
    nc = _get_program()
    res = run_bass_kernel_spmd(nc, in_maps, core_ids=list(range(NCORES)))
    outs = [res.results[c]["out"].reshape(T, BC, N) for c in range(NCORES)]
    return np.concatenate(outs, axis=1)



# revision 19
# speedup vs baseline: 1.0085x; 1.0085x over previous
"""Trainium2 Bass kernel: receptive-field one-hot time encoder.

reference semantics:
    t_spike[n, b] = clip(int(scaling[n] * |x[b] - center[n]|), 0, T-1)
    out[t, b, n]  = 1.0 if t == t_spike[n, b] else 0.0       # [T, B, N] f32

Data-parallel over 8 cores: x and the output batch axis are sharded into 8
contiguous blocks of B_c = 16384; center/scaling are replicated.

Per-core layout: x viewed as [128 partitions, 128 cols]; all per-(b, n)
quantities live in [128, 2048] tiles with free index f*16+n, which makes each
output t-slice a single fully contiguous 1 MiB DRAM store ([T, B_c, N] with
b = p*128+f).

The kernel is DMA-write-bound: 64 MiB of output per core at 360 GB/s is
~186.4 us, so everything else is organized to keep the DMA engines saturated
from the earliest possible instant:

  - one small input DMA (center+scaling+x) lands ASAP;
  - an all-DVE prep chain (no cross-engine hops):
        diff = x - center                  (tensor_tensor, broadcast APs)
        u    = max(diff, -diff) = |diff|   (tensor_scalar mult -1 + tensor_tensor max)
        v    = u * scaling                 (tensor_tensor)
        r    = (v + 2^23) - 2^23           (dual-op tensor_scalar: nearest int)
        tsp  = r - (r > v)                 (exact floor for 0 <= v < 2^23 under
                                            any nearest rounding of the add)
    t=0 one-hot = is_lt(v, 1), t=63 = is_ge(v, 63), inner t = is_equal(tsp,t);
  - x-columns processed in a ramp of growing chunks; chunk c emits one-hot
    row pieces for t = 0, 63, 1..RAMPS[c] over its own columns, with the next
    chunk's prep ops interleaved between emissions so the DMA backlog never
    drains during prep bursts;
  - rows RAMPS[c] < t <= RAMPS[c-1] get a single complement piece covering
    the later chunks' columns once the full tsp tile exists, then rows
    t > RAMPS[0] go out as single full-width DMAs;
  - early DMAs alternate between the SP/Act HWDGE paths and the Pool SWDGE
    path so descriptor generation never starves the DMA engines.

The compares are exact: v is an exact f32 product, mod(v,1) the exact
remainder, so tsp is the exact floor and outputs match the jax reference
bit-for-bit (relative error 0.0).
"""

import sys

if "/opt/trn_rl_repo" not in sys.path:
    sys.path.insert(0, "/opt/trn_rl_repo")

import numpy as np

import concourse.tile as tile
from concourse import bacc, mybir
from concourse.alu_op_type import AluOpType
from concourse.bass_utils import run_bass_kernel_spmd

B, N, T = 131072, 16, 64
NCORES = 8
BC = B // NCORES  # 16384 per core
P = 128
F = BC // P  # 128 x-columns per partition
FN = F * N  # 2048 free elems per output row

# --- tunables -------------------------------------------------------------
CHUNK_COLS = [16, 24, 32, 56]   # x-column ramp (must sum to F)
RAMPS = [14, 10, 6, 2]          # per-chunk ramp rows t=1..RAMPS[c] (decreasing)
INTERLEAVE = 2                  # ramp emissions between interleaved prep ops
OUT_BUFS = 16
IN_SPLIT = 16                   # None = one input DMA; else split after this many x cols
SHARE_TAGS = True               # share prep-tile tags across chunks (WAR serialization)
PP_BUFS = 1                     # ring depth of the prep-tile pool
WAIT_HINTS = [None, 0.005, 0.009, 0.013]  # per-chunk scheduler wait hint (ms)
PREP_ORDER = ["diff", "neg", "u", "v", "r", "o0", "corr", "o63", "tsp"]  # chunk-prep emission order
ABS_MODE = "act"                # "dve": max(diff,-diff); "act": ACT Abs
THR_MODE = "v"                  # "v": o0/o63 = tensor_scalar on v;
                                # "u": tensor_tensor vs 1/s, 63/s thresholds
                                #      (bit-exact when scaling is a power of 2)
EARLY_ENGINES = ["pool", "sp", "act"]   # issue ring for chunk-piece DMAs
STEADY_ENGINES = ["sp", "act"]          # issue ring for complement/full rows

f32 = mybir.dt.float32

_cached = {}


def build_program():
    nc = bacc.Bacc(
        "TRN2", target_bir_lowering=False, debug=False, enable_asserts=False
    )
    # packed input: [:, :N]=center, [:, N:2N]=scaling,
    # then (u-mode only) [:, 2N:3N]=1/scaling, [:, 3N:4N]=63/scaling, then x
    npar = 4 * N if THR_MODE == "u" else 2 * N
    pk_d = nc.dram_tensor("packed", [P, npar + F], f32, kind="ExternalInput").ap()
    out_d = nc.dram_tensor("out", [T, P, FN], f32, kind="ExternalOutput").ap()

    nchunks = len(CHUNK_COLS)
    po = list(PREP_ORDER)
    need_after = {
        "neg": ["diff"], "u": ["diff"] + (["neg"] if ABS_MODE == "dve" else []),
        "v": ["u"], "r": ["v"], "corr": ["r", "v"], "tsp": ["r", "corr"],
        "o0": ["u"] if THR_MODE == "u" else ["v"],
        "o63": ["u"] if THR_MODE == "u" else ["v"],
    }
    for k, deps in need_after.items():
        for d in deps:
            assert po.index(k) > po.index(d), f"PREP_ORDER: {k} before {d}"
    assert sum(CHUNK_COLS) == F
    assert len(RAMPS) == nchunks
    assert all(RAMPS[i] >= RAMPS[i + 1] for i in range(nchunks - 1))
    starts = [sum(CHUNK_COLS[:i]) for i in range(nchunks)]

    with tile.TileContext(nc) as tc:
        with (
            tc.tile_pool(name="prep", bufs=1) as prep,
            tc.tile_pool(name="pp", bufs=PP_BUFS) as pp,
            tc.tile_pool(name="outp", bufs=OUT_BUFS) as outp,
        ):
            pk_sb = prep.tile([P, npar + F], f32)
            v = prep.tile([P, FN], f32)
            tsp = prep.tile([P, FN], f32)

            if IN_SPLIT is None:
                nc.sync.dma_start(pk_sb[:], pk_d[:])
            else:
                cut = npar + IN_SPLIT
                nc.sync.dma_start(pk_sb[:, :cut], pk_d[:, :cut])
                nc.scalar.dma_start(pk_sb[:, cut:], pk_d[:, cut:])

            thr0 = pk_sb[:, 2 * N : 3 * N] if THR_MODE == "u" else None
            thr63 = pk_sb[:, 3 * N : 4 * N] if THR_MODE == "u" else None

            eng_map = {"sp": nc.sync, "act": nc.scalar, "pool": nc.gpsimd}
            eidx = {"early": [0], "steady": [0]}

            def dma(kind, dst, src):
                ring = EARLY_ENGINES if kind == "early" else STEADY_ENGINES
                i = eidx[kind]
                eng_map[ring[i[0] % len(ring)]].dma_start(dst, src)
                i[0] += 1

            def bcast(ap, w):
                return ap.rearrange("p (f n) -> p f n", f=1).broadcast_to([P, w, N])

            def onehot_piece(t, ws, wc, kind, u_tile=None):
                o = outp.tile([P, FN], f32, tag="onehot")
                if t == 0 and u_tile is not None:
                    nc.vector.tensor_tensor(
                        o[:, :wc].rearrange("p (f n) -> p f n", n=N),
                        u_tile[:].rearrange("p (f n) -> p f n", n=N),
                        bcast(thr0, wc // N),
                        op=AluOpType.is_lt,
                    )
                elif t == T - 1 and u_tile is not None:
                    nc.vector.tensor_tensor(
                        o[:, :wc].rearrange("p (f n) -> p f n", n=N),
                        u_tile[:].rearrange("p (f n) -> p f n", n=N),
                        bcast(thr63, wc // N),
                        op=AluOpType.is_ge,
                    )
                elif t == 0:
                    nc.vector.tensor_scalar(
                        o[:, :wc], v[:, ws], 1.0, None, op0=AluOpType.is_lt
                    )
                elif t == T - 1:
                    nc.vector.tensor_scalar(
                        o[:, :wc], v[:, ws], float(T - 1), None,
                        op0=AluOpType.is_ge,
                    )
                else:
                    nc.vector.tensor_scalar(
                        o[:, :wc], tsp[:, ws], float(t), None,
                        op0=AluOpType.is_equal,
                    )
                dma(kind, out_d[t, :, ws], o[:, :wc])

            def prep_ops(c):
                """Closures for chunk c's prep: diff, u, v, o0, o63, vm, tsp."""
                col, w = starts[c], CHUNK_COLS[c]
                e0, e1 = col * N, (col + w) * N
                wc = w * N
                ws = slice(e0, e1)
                x_b = pk_sb[:, npar + col : npar + col + w].broadcast_to(
                    [P, w, N]
                )
                cen_b = bcast(pk_sb[:, 0:N], w)
                sca_b = bcast(pk_sb[:, N : 2 * N], w)
                sfx = "" if SHARE_TAGS else str(c)
                diff = pp.tile([P, wc], f32, tag=f"diff{sfx}")
                neg = (pp.tile([P, wc], f32, tag=f"neg{sfx}")
                       if ABS_MODE == "dve" else None)
                u = pp.tile([P, wc], f32, tag=f"u{sfx}")
                r = pp.tile([P, wc], f32, tag=f"r{sfx}")
                corr = pp.tile([P, wc], f32, tag=f"corr{sfx}")
                ops = {
                    "diff": lambda: nc.vector.tensor_tensor(
                        diff[:].rearrange("p (f n) -> p f n", n=N),
                        x_b, cen_b, op=AluOpType.subtract,
                    ),
                    "neg": lambda: (
                        nc.vector.tensor_scalar(
                            neg[:], diff[:], -1.0, None, op0=AluOpType.mult
                        ) if ABS_MODE == "dve" else None
                    ),
                    "u": lambda: (
                        nc.vector.tensor_tensor(
                            u[:], diff[:], neg[:], op=AluOpType.max
                        ) if ABS_MODE == "dve" else
                        nc.scalar.activation(
                            u[:], diff[:], mybir.ActivationFunctionType.Abs
                        )
                    ),
                    "v": lambda: nc.vector.tensor_tensor(
                        v[:, ws].rearrange("p (f n) -> p f n", n=N),
                        u[:].rearrange("p (f n) -> p f n", n=N),
                        sca_b, op=AluOpType.mult,
                    ),
                    "o0": lambda: onehot_piece(
                        0, ws, wc, "early",
                        u_tile=u if THR_MODE == "u" else None,
                    ),
                    "o63": lambda: onehot_piece(
                        T - 1, ws, wc, "early",
                        u_tile=u if THR_MODE == "u" else None,
                    ),
                    "r": lambda: nc.vector.tensor_scalar(
                        r[:], v[:, ws], 8388608.0, 8388608.0,
                        op0=AluOpType.add, op1=AluOpType.subtract,
                    ),
                    "corr": lambda: nc.vector.tensor_tensor(
                        corr[:], r[:], v[:, ws], op=AluOpType.is_gt
                    ),
                    "tsp": lambda: nc.vector.tensor_tensor(
                        tsp[:, ws], r[:], corr[:], op=AluOpType.subtract
                    ),
                }
                return [ops[k] for k in PREP_ORDER]

            def ramp_ops(c):
                col, w = starts[c], CHUNK_COLS[c]
                e0, e1 = col * N, (col + w) * N
                ws = slice(e0, e1)
                return [
                    (lambda t=t: onehot_piece(t, ws, w * N, "early"))
                    for t in range(1, 1 + RAMPS[c])
                ]

            # chunk 0 prep, then ramps of chunk c interleaved with prep of c+1
            for op in prep_ops(0):
                op()
            for c in range(nchunks):
                ramps = ramp_ops(c)
                nxt = prep_ops(c + 1) if c + 1 < nchunks else []
                ri = 0
                hint = WAIT_HINTS[c + 1] if WAIT_HINTS and c + 1 < nchunks else None
                def emit_next():
                    if hint is None:
                        nxt.pop(0)()
                    else:
                        with tc.tile_wait_until(hint):
                            nxt.pop(0)()
                for op in ramps:
                    op()
                    ri += 1
                    if ri % INTERLEAVE == 0 and nxt:
                        emit_next()
                while nxt:
                    emit_next()

            # complement pieces for ramp rows + full-width rows, ascending t
            for t in range(1 + RAMPS[-1], T - 1):
                covered = sum(
                    CHUNK_COLS[c] for c in range(nchunks) if RAMPS[c] >= t
                )
                e0 = covered * N
                if e0 >= FN:
                    continue
                onehot_piece(t, slice(e0, FN), FN - e0, "steady")

    nc.compile()
    return nc


def _get_program():
    if "nc" not in _cached:
        _cached["nc"] = build_program()
    return _cached["nc"]


def kernel(x, center, scaling, time_steps):
    assert int(time_steps) == T
    x = np.ascontiguousarray(np.asarray(x, dtype=np.float32)).reshape(
        NCORES, P, F
    )
    cen = np.asarray(center, np.float32).reshape(N)
    sca = np.asarray(scaling, np.float32).reshape(N)
    npar = 4 * N if THR_MODE == "u" else 2 * N
    packed = np.empty((NCORES, P, npar + F), np.float32)
    packed[:, :, 0:N] = cen
    packed[:, :, N : 2 * N] = sca
    if THR_MODE == "u":
        packed[:, :, 2 * N : 3 * N] = np.float32(1.0) / sca
        packed[:, :, 3 * N : 4 * N] = np.float32(T - 1) / sca
    packed[:, :, npar :] = x
    in_maps = [{"packed": packed[c]} for c in range(NCORES)]
    nc = _get_program()
    res = run_bass_kernel_spmd(nc, in_maps, core_ids=list(range(NCORES)))
    outs = [res.results[c]["out"].reshape(T, BC, N) for c in range(NCORES)]
    return np.concatenate(outs, axis=1)
